# revision 1
# baseline (speedup 1.0000x reference)
"""SC-LSTM decoder (2-layer, teacher-forced) Trainium2 Bass kernel.

Strategy (8 NeuronCores):
  - Tensor-parallel over the hidden dimension: core j owns H-rows
    [128j, 128j+128) of each layer (and V-cols [256j, 256j+256) of the
    output projection). Full batch B=128 stays on every core, which
    exactly fills the PE stationary dimension.
  - Phase A (parallel): precompute all x-dependent GEMM contributions
    (gx0 = x@w2h_W0, gx1 = x@w2h_W1[:E], rx = x@w2hr) for all T steps,
    column-sharded across cores.  X is pre-transposed on the host so the
    contraction dim (E) lands on SBUF partitions.
  - Phase C (sequential over T): per step, each core computes its H-slice
    of the gates from SBUF-resident weight slices, the cell update, and
    its new hidden slice; two small AllGathers (64KB/rank) rebuild the
    full transposed hidden state h^T for the next step's contractions.
    The output projection slice runs inside the loop off the gathered
    h^T tiles.
"""

import sys

sys.path.insert(0, "/opt/trn_rl_repo")

import numpy as np

import concourse.bass as bass
import concourse.mybir as mybir
import concourse.tile as tile
from concourse import bacc
from concourse.bass_utils import run_bass_kernel_spmd
from concourse.masks import make_identity

B, T, E, H, D, V, L = 128, 100, 2048, 1024, 256, 2048, 2
NC = 8
P = 128
HS = H // NC      # 128 h-rows per core per layer
GS = 4 * HS       # 512 packed gate cols per core
VS = V // NC      # 256 output cols per core
KE = E // P       # 16 k-tiles over E
KH = H // P       # 8 k-tiles over H
F32 = mybir.dt.float32
F32R = mybir.dt.float32r

_cache = {}


def _build(t_steps: int):
    nc = bacc.Bacc("TRN2", target_bir_lowering=False, debug=False, num_devices=NC)
    BF16 = mybir.dt.bfloat16
    LA = 2  # gx lookahead: gx(t+LA) computed inside step t's AG windows

    # ---------------- I/O declarations (per-core values supplied via in_maps)
    xT = nc.dram_tensor("xT", [E, t_steps * B], F32R, kind="ExternalInput")
    h0T_i = nc.dram_tensor("h0T_i", [H, B], F32R, kind="ExternalInput")
    c_i = nc.dram_tensor("c_i", [B, HS], F32, kind="ExternalInput")
    d_i = nc.dram_tensor("d_i", [B, D], F32, kind="ExternalInput")
    Wx0 = nc.dram_tensor("Wx0", [E, GS], F32R, kind="ExternalInput")
    Wx1x = nc.dram_tensor("Wx1x", [E, GS], F32R, kind="ExternalInput")
    Wrx = nc.dram_tensor("Wrx", [E, 2 * D], F32R, kind="ExternalInput")
    Wh0 = nc.dram_tensor("Wh0", [H, GS], F32R, kind="ExternalInput")
    Wh1 = nc.dram_tensor("Wh1", [H, GS], F32R, kind="ExternalInput")
    Wx1h = nc.dram_tensor("Wx1h", [H, GS], F32R, kind="ExternalInput")
    Wrc = nc.dram_tensor("Wrc", [2 * H, D], F32R, kind="ExternalInput")
    Wr1h = nc.dram_tensor("Wr1h", [H, D], F32R, kind="ExternalInput")
    Wdc0 = nc.dram_tensor("Wdc0", [D, HS], F32R, kind="ExternalInput")
    Wdc1 = nc.dram_tensor("Wdc1", [D, HS], F32R, kind="ExternalInput")
    Wout = nc.dram_tensor("Wout", [2 * H, VS], F32R, kind="ExternalInput")

    out_o = nc.dram_tensor("out", [t_steps, B, VS], F32, kind="ExternalOutput")

    # DRAM scratch for the precomputed r-gate x-contributions
    rxd = nc.dram_tensor("rxd", [t_steps, B, 2 * D], F32)
    gx1d = nc.dram_tensor("gx1d", [t_steps, B, GS], F32)

    rg = [list(range(NC))]

    with tile.TileContext(nc) as tc:
        with tc.tile_pool(name="const", bufs=1) as constp:
            ident = constp.tile([P, P], F32)
            make_identity(nc, ident[:])

            # ---------------- Phase A: precompute rx (r-gate x contributions)
            with (
                tc.tile_pool(name="wxa", bufs=1) as wxap,
                tc.tile_pool(name="xa", bufs=3) as xap,
                tc.tile_pool(name="ga", bufs=3) as gap,
                tc.tile_pool(name="psa", bufs=2, space="PSUM") as psa,
            ):
                wrx = wxap.tile([P, KE, 2 * D], F32R)
                wx1a = wxap.tile([P, KE, GS], F32R)
                nc.sync.dma_start(wrx[:], Wrx.rearrange("(k p) n -> p k n", p=P))
                nc.sync.dma_start(wx1a[:], Wx1x.rearrange("(k p) n -> p k n", p=P))
                for t in range(t_steps):
                    xt = xap.tile([P, KE, B], F32R, tag="xt", name="xt")
                    nc.sync.dma_start(
                        xt[:],
                        xT[:, t * B : (t + 1) * B].rearrange("(k p) n -> p k n", p=P),
                    )
                    rxp = psa.tile([B, 2 * D], F32, tag="rxp", bufs=2, name="rxp")
                    g1xp = psa.tile([B, GS], F32, tag="g1xp", bufs=2, name="g1xp")
                    for k in range(KE):
                        st, sp = (k == 0), (k == KE - 1)
                        nc.tensor.matmul(rxp[:], xt[:, k, :], wrx[:, k, :], start=st, stop=sp)
                        nc.tensor.matmul(g1xp[:], xt[:, k, :], wx1a[:, k, :], start=st, stop=sp)
                    rxs = gap.tile([B, 2 * D], F32, tag="rxs", name="rxs")
                    nc.vector.tensor_copy(rxs[:], rxp[:])
                    nc.sync.dma_start(rxd[t], rxs[:])
                    g1xs = gap.tile([B, GS], F32, tag="g1xs", name="g1xs")
                    nc.vector.tensor_copy(g1xs[:], g1xp[:])
                    nc.sync.dma_start(gx1d[t], g1xs[:])

            # ---------------- Phase B/C: recurrence with interleaved gx GEMMs
            with (
                tc.tile_pool(name="wr", bufs=1) as wrp,
                tc.tile_pool(name="st", bufs=2) as stp,
                tc.tile_pool(name="gx", bufs=3) as gxp,
                tc.tile_pool(name="wk", bufs=2) as wkp,
                tc.tile_pool(name="psg", bufs=2, space="PSUM") as psg,
                tc.tile_pool(name="psr", bufs=2, space="PSUM") as psr,
                tc.tile_pool(name="pst", bufs=2, space="PSUM") as pst,
                tc.tile_pool(name="dma_b", bufs=4, space="DRAM") as dramp,
            ):
                wh0 = wrp.tile([P, KH, GS], F32R)
                wh1 = wrp.tile([P, KH, GS], F32R)
                wx1h = wrp.tile([P, KH, GS], F32R)
                wrc = wrp.tile([P, 2 * KH, D], F32R)
                wr1h = wrp.tile([P, KH, D], F32R)
                wdc0 = wrp.tile([P, D // P, HS], F32R)
                wdc1 = wrp.tile([P, D // P, HS], F32R)
                wout = wrp.tile([P, 2 * KH, VS], F32R)
                wx0 = wrp.tile([P, KE, GS], F32R)
                nc.sync.dma_start(wh0[:], Wh0.rearrange("(k p) n -> p k n", p=P))
                nc.sync.dma_start(wh1[:], Wh1.rearrange("(k p) n -> p k n", p=P))
                nc.sync.dma_start(wx1h[:], Wx1h.rearrange("(k p) n -> p k n", p=P))
                nc.sync.dma_start(wrc[:], Wrc.rearrange("(k p) n -> p k n", p=P))
                nc.sync.dma_start(wr1h[:], Wr1h.rearrange("(k p) n -> p k n", p=P))
                nc.sync.dma_start(wdc0[:], Wdc0.rearrange("(k p) n -> p k n", p=P))
                nc.sync.dma_start(wdc1[:], Wdc1.rearrange("(k p) n -> p k n", p=P))
                nc.sync.dma_start(wout[:], Wout.rearrange("(k p) n -> p k n", p=P))
                nc.sync.dma_start(wx0[:], Wx0.rearrange("(k p) n -> p k n", p=P))

                h0T = stp.tile([P, KH, B], F32R, tag="h0T", name="h0Ti")
                h1T = stp.tile([P, KH, B], F32R, tag="h1T", name="h1Ti")
                nc.sync.dma_start(h0T[:], h0T_i.rearrange("(k p) n -> p k n", p=P))
                nc.sync.dma_start(h1T[:], h0T_i.rearrange("(k p) n -> p k n", p=P))
                c0 = stp.tile([B, HS], F32, tag="c0", name="c0i")
                c1 = stp.tile([B, HS], F32, tag="c1", name="c1i")
                nc.sync.dma_start(c0[:], c_i[:])
                nc.sync.dma_start(c1[:], c_i[:])
                d0 = stp.tile([B, D], F32, tag="d0", name="d0i")
                d1 = stp.tile([B, D], F32, tag="d1", name="d1i")
                nc.sync.dma_start(d0[:], d_i[:])
                nc.sync.dma_start(d1[:], d_i[:])

                Sig = mybir.ActivationFunctionType.Sigmoid
                Tanh = mybir.ActivationFunctionType.Tanh
                mul = mybir.AluOpType.mult
                add = mybir.AluOpType.add

                def load_xt(u):
                    xtb = gxp.tile([P, KE, B], F32R, tag="xtb", bufs=2, name="xtb")
                    nc.sync.dma_start(
                        xtb[:],
                        xT[:, u * B : (u + 1) * B].rearrange("(k p) n -> p k n", p=P),
                    )
                    return xtb

                def gx_compute(xtb, pin_after=None):
                    """In-loop x-contribution GEMMs (bf16) — AG-window filler."""
                    g0x = psr.tile([B, GS], F32, tag="rc0p", bufs=1, name="g0xp")
                    for k in range(KE):
                        m = nc.tensor.matmul(g0x[:], xtb[:, k, :], wx0[:, k, :], start=(k == 0), stop=(k == KE - 1))
                        if k == 0 and pin_after is not None:
                            bass._add_dep_helper(m.ins, pin_after.ins, sync=True, reason="pin filler into AG window")
                    gx0 = gxp.tile([B, GS], F32, tag="gx0", name="gx0")
                    nc.vector.tensor_copy(gx0[:], g0x[:])
                    return gx0

                def gate_act(gp_ap, gx, li, c_cur):
                    gsum = wkp.tile([B, GS], F32, tag=f"gsum{li}", bufs=1, name=f"gsum{li}")
                    nc.vector.tensor_tensor(gsum[:], gp_ap, gx[:], add)
                    sig = wkp.tile([B, 3 * HS], F32, tag=f"sig{li}", bufs=1, name=f"sig{li}")
                    nc.scalar.activation(sig[:], gsum[:, : 3 * HS], Sig)
                    tgc = wkp.tile([B, HS], F32, tag=f"tgc{li}", name=f"tgc{li}")
                    nc.scalar.activation(tgc[:], gsum[:, 3 * HS :], Tanh)
                    cpart = wkp.tile([B, HS], F32, tag=f"cpart{li}", name=f"cpart{li}")
                    nc.vector.tensor_tensor(cpart[:], sig[:, :HS], tgc[:], mul)
                    m2 = wkp.tile([B, HS], F32, tag=f"m2{li}", name=f"m2{li}")
                    nc.vector.tensor_tensor(m2[:], sig[:, HS : 2 * HS], c_cur[:], mul)
                    nc.vector.tensor_tensor(cpart[:], cpart[:], m2[:], add)
                    return gsum, sig, cpart

                def r_dc_path(rsum_in, extra_ps, d_cur, wdc, li):
                    rs = wkp.tile([B, D], F32, tag=f"rsum{li}", bufs=1, name=f"rsum{li}")
                    if extra_ps is not None:
                        nc.vector.tensor_tensor(rs[:], rsum_in, extra_ps, add)
                        nc.scalar.activation(rs[:], rs[:], Sig)
                    else:
                        nc.scalar.activation(rs[:], rsum_in, Sig)
                    d_new = stp.tile([B, D], F32, tag=f"d{li}", name=f"d{li}")
                    nc.vector.tensor_tensor(d_new[:], rs[:], d_cur[:], mul)
                    dtT_p = pst.tile([P, D // P, B], F32, tag="dtTp", bufs=1, name=f"dtTp{li}")
                    for k in range(D // P):
                        nc.tensor.transpose(dtT_p[:, k, :], d_new[:, k * P : (k + 1) * P], ident[:])
                    dtT = wkp.tile([P, D // P, B], F32R, tag=f"dtT{li}", bufs=1, name=f"dtT{li}")
                    nc.vector.tensor_copy(dtT[:], dtT_p[:])
                    dcp = psr.tile([B, HS], F32, tag="dcp", bufs=1, name=f"dcp{li}")
                    for k in range(D // P):
                        nc.tensor.matmul(dcp[:], dtT[:, k, :], wdc[:, k, :], start=(k == 0), stop=(k == D // P - 1))
                    tdc = wkp.tile([B, HS], F32, tag=f"tdc{li}", name=f"tdc{li}")
                    nc.scalar.activation(tdc[:], dcp[:], Tanh)
                    return tdc, d_new

                def finish_cell(cpart, tdc, sig, li):
                    c_new = stp.tile([B, HS], F32, tag=f"c{li}", name=f"c{li}")
                    nc.vector.tensor_tensor(c_new[:], cpart[:], tdc[:], add)
                    nh = wkp.tile([B, HS], F32, tag=f"nh{li}", name=f"nh{li}")
                    nc.scalar.activation(nh[:], c_new[:], Tanh)
                    nc.vector.tensor_tensor(nh[:], sig[:, 2 * HS : 3 * HS], nh[:], mul)
                    return nh, c_new

                def trigger_gather(nh, li):
                    nhT_p = pst.tile([P, B], F32, tag="nhTp", bufs=1, name=f"nhTp{li}")
                    nc.tensor.transpose(nhT_p[:], nh[:], ident[:])
                    nhT = wkp.tile([P, B], F32R, tag=f"nhT{li}", name=f"nhT{li}")
                    nc.vector.tensor_copy(nhT[:], nhT_p[:])
                    agi = dramp.tile([P, B], F32R, tag=f"agi{li}", name=f"agi{li}")
                    ago = dramp.tile([H, B], F32R, tag=f"ago{li}", addr_space="Shared", name=f"ago{li}")
                    nc.sync.dma_start(agi[:], nhT[:])
                    cc = nc.gpsimd.collective_compute(
                        "AllGather", mybir.AluOpType.bypass, replica_groups=rg,
                        ins=[agi[:]], outs=[ago[:]],
                    )
                    return ago, cc

                def load_gathered(ago, li):
                    hT_new = stp.tile([P, KH, B], F32R, tag=f"h{li}T", name=f"h{li}T")
                    nc.sync.dma_start(
                        hT_new[:, : KH // 2, :],
                        ago[: H // 2, :].rearrange("(k p) n -> p k n", p=P),
                    )
                    nc.sync.dma_start(
                        hT_new[:, KH // 2 :, :],
                        ago[H // 2 :, :].rearrange("(k p) n -> p k n", p=P),
                    )
                    return hT_new

                # ---------------- prologue: initial rc parts + gx for steps 0..LA
                rc0p = psr.tile([B, D], F32, tag="rc0p", bufs=1, name="rc0pi")
                for k in range(KH):
                    nc.tensor.matmul(rc0p[:], h0T[:, k, :], wrc[:, k, :], start=(k == 0), stop=(k == KH - 1))
                rc1p = psr.tile([B, D], F32, tag="rc1p", bufs=1, name="rc1pi")
                for k in range(KH):
                    nc.tensor.matmul(rc1p[:], h1T[:, k, :], wrc[:, KH + k, :], start=(k == 0), stop=(k == KH - 1))
                gx_ring = [None] * (LA + 1)
                for u in range(min(LA, t_steps)):
                    gx_ring[u] = gx_compute(load_xt(u))

                out_prev = None
                for t in range(t_steps):
                    rx = gxp.tile([B, 2 * D], F32, tag="rx", bufs=2, name="rx")
                    nc.sync.dma_start(rx[:], rxd[t])
                    gx1 = gxp.tile([B, GS], F32, tag="gx1", bufs=2, name="gx1")
                    nc.sync.dma_start(gx1[:], gx1d[t])
                    gx0 = gx_ring[t % (LA + 1)]

                    # ---- g0(t) + g1 h1-part: tail fillers of the AG#2(t-1) window
                    g0p = psg.tile([B, GS], F32, tag="gout", bufs=1, name="g0p")
                    for k in range(KH):
                        nc.tensor.matmul(g0p[:], h0T[:, k, :], wh0[:, k, :], start=(k == 0), stop=(k == KH - 1))
                    # rc1-part(t): first AG#2(t-1)-dependent PE work
                    rc1p = psr.tile([B, D], F32, tag="rc1p", bufs=1, name="rc1p")
                    for k in range(KH):
                        nc.tensor.matmul(rc1p[:], h1T[:, k, :], wrc[:, KH + k, :], start=(k == 0), stop=(k == KH - 1))
                    g1p = psg.tile([B, GS], F32, tag="g1p", bufs=1, name="g1p")
                    for k in range(KH):
                        nc.tensor.matmul(g1p[:], h1T[:, k, :], wh1[:, k, :], start=(k == 0), stop=False)

                    rpre0 = wkp.tile([B, D], F32, tag="rpre0", bufs=1, name="rpre0")
                    nc.vector.tensor_tensor(rpre0[:], rc0p[:], rx[:, :D], add)
                    rpre1 = wkp.tile([B, D], F32, tag="rpre1", bufs=1, name="rpre1")
                    nc.vector.tensor_tensor(rpre1[:], rc0p[:], rx[:, D:], add)
                    nc.vector.tensor_tensor(rpre0[:], rpre0[:], rc1p[:], add)
                    nc.vector.tensor_tensor(rpre1[:], rpre1[:], rc1p[:], add)

                    gsum0, sig0, cpart0 = gate_act(g0p[:], gx0, 0, c0)
                    tdc0, d0 = r_dc_path(rpre0[:], None, d0, wdc0, 0)
                    nh0, c0 = finish_cell(cpart0, tdc0, sig0, 0)
                    ago0, cc0 = trigger_gather(nh0, 0)

                    # ---- AG#1(t) window fillers: gx(t+LA) + out(t-1)
                    if t + LA < t_steps:
                        gx_ring[(t + LA) % (LA + 1)] = gx_compute(load_xt(t + LA), pin_after=cc0)
                    if out_prev is not None:
                        po0, po1, pt = out_prev
                        outp = psg.tile([B, VS], F32, tag="gout", bufs=1, name="outp")
                        for k in range(2 * KH):
                            src = po0[:, k, :] if k < KH else po1[:, k - KH, :]
                            nc.tensor.matmul(outp[:], src, wout[:, k, :], start=(k == 0), stop=(k == 2 * KH - 1))
                        osb = wkp.tile([B, VS], F32, tag="osb", name="osb")
                        nc.vector.tensor_copy(osb[:], outp[:])
                        nc.sync.dma_start(out_o[pt], osb[:])

                    h0T_new = load_gathered(ago0, 0)

                    # ---- post-AG#1 spine: r1-part + g1 nh0-part
                    r1p = psr.tile([B, D], F32, tag="r1p", bufs=1, name="r1p")
                    for k in range(KH):
                        nc.tensor.matmul(r1p[:], h0T_new[:, k, :], wr1h[:, k, :], start=(k == 0), stop=(k == KH - 1))
                    for k in range(KH):
                        nc.tensor.matmul(g1p[:], h0T_new[:, k, :], wx1h[:, k, :], start=False, stop=(k == KH - 1))

                    gsum1, sig1, cpart1 = gate_act(g1p[:], gx1, 1, c1)
                    tdc1, d1 = r_dc_path(rpre1[:], r1p[:], d1, wdc1, 1)
                    nh1, c1 = finish_cell(cpart1, tdc1, sig1, 1)
                    ago1, cc1 = trigger_gather(nh1, 1)

                    # ---- AG#2(t) window filler: rc0-part(t+1); g0(t+1)/rc1p(t+1)
                    # continue the fill at the top of the next iteration
                    h1T_new = load_gathered(ago1, 1)
                    rc0p = psr.tile([B, D], F32, tag="rc0p", bufs=1, name="rc0p")
                    for k in range(KH):
                        m = nc.tensor.matmul(rc0p[:], h0T_new[:, k, :], wrc[:, k, :], start=(k == 0), stop=(k == KH - 1))
                        if k == 0:
                            bass._add_dep_helper(m.ins, cc1.ins, sync=True, reason="pin filler into AG window")

                    out_prev = (h0T_new, h1T_new, t)
                    h0T, h1T = h0T_new, h1T_new

                po0, po1, pt = out_prev
                outp = psg.tile([B, VS], F32, tag="gout", bufs=1, name="outpF")
                for k in range(2 * KH):
                    src = po0[:, k, :] if k < KH else po1[:, k - KH, :]
                    nc.tensor.matmul(outp[:], src, wout[:, k, :], start=(k == 0), stop=(k == 2 * KH - 1))
                osb = wkp.tile([B, VS], F32, tag="osb", name="osbF")
                nc.vector.tensor_copy(osb[:], outp[:])
                nc.sync.dma_start(out_o[pt], osb[:])

    nc.compile()
    return nc


def _prep_inputs(input_seq, h0, dt0, w2h_W0, w2h_b0, w2h_W1, w2h_b1,
                 w2hr_W0, w2hr_b0, w2hr_W1, w2hr_b1,
                 h2h_W0, h2h_b0, h2h_W1, h2h_b1,
                 h2hr_W0, h2hr_b0, h2hr_W1, h2hr_b1,
                 dc_W0, dc_W1, out_W, out_b, t_steps):
    f = np.float32
    for name, b in [("w2h_b0", w2h_b0), ("w2h_b1", w2h_b1), ("w2hr_b0", w2hr_b0),
                    ("w2hr_b1", w2hr_b1), ("h2h_b0", h2h_b0), ("h2h_b1", h2h_b1),
                    ("h2hr_b0", h2hr_b0), ("h2hr_b1", h2hr_b1), ("out_b", out_b)]:
        assert not np.any(np.asarray(b)), f"nonzero bias {name} unsupported"

    # time-step inputs: SOS one-hot at t=0, then input_seq[:, t-1]
    xs = np.empty((t_steps, B, E), f)
    xs[0] = 0.0
    xs[0, :, 0] = 1.0
    xs[1:] = np.asarray(input_seq, f).transpose(1, 0, 2)[: t_steps - 1]
    xT = np.ascontiguousarray(xs.reshape(t_steps * B, E).T)

    h0 = np.asarray(h0, f)
    h0T = np.ascontiguousarray(h0.T)
    dt0 = np.asarray(dt0, f)

    alpha = 1.0 / L
    wrc_full = np.concatenate([np.asarray(h2hr_W0, f), np.asarray(h2hr_W1, f)], 0) * alpha
    wrx_full = np.concatenate([np.asarray(w2hr_W0, f), np.asarray(w2hr_W1, f)[:E]], 1)

    in_maps = []
    for j in range(NC):
        gc = np.r_[tuple(np.arange(g * H + j * HS, g * H + (j + 1) * HS) for g in range(4))]
        vs = slice(j * VS, (j + 1) * VS)
        in_maps.append({
            "xT": xT,
            "h0T_i": h0T,
            "c_i": np.ascontiguousarray(h0[:, j * HS : (j + 1) * HS]),
            "d_i": dt0,
            "Wx0": np.ascontiguousarray(np.asarray(w2h_W0, f)[:, gc]),
            "Wx1x": np.ascontiguousarray(np.asarray(w2h_W1, f)[:E, gc]),
            "Wrx": wrx_full,
            "Wh0": np.ascontiguousarray(np.asarray(h2h_W0, f)[:, gc]),
            "Wh1": np.ascontiguousarray(np.asarray(h2h_W1, f)[:, gc]),
            "Wx1h": np.ascontiguousarray(np.asarray(w2h_W1, f)[E:, gc]),
            "Wrc": wrc_full,
            "Wr1h": np.ascontiguousarray(np.asarray(w2hr_W1, f)[E:]),
            "Wdc0": np.ascontiguousarray(np.asarray(dc_W0, f)[:, j * HS : (j + 1) * HS]),
            "Wdc1": np.ascontiguousarray(np.asarray(dc_W1, f)[:, j * HS : (j + 1) * HS]),
            "Wout": np.ascontiguousarray(np.asarray(out_W, f)[:, vs]),
        })
    return in_maps


def _run(t_steps, trace, **inputs):
    if trace:
        import prof_shim

        prof_shim.install()
    key = t_steps
    if key not in _cache:
        _cache[key] = _build(t_steps)
    nc = _cache[key]
    in_maps = _prep_inputs(**inputs, t_steps=t_steps)
    res = run_bass_kernel_spmd(nc, in_maps, list(range(NC)), trace=trace)
    parts = [res.results[j]["out"] for j in range(NC)]  # each (T, B, VS)
    full = np.concatenate(parts, axis=2)                # (T, B, V)
    return np.ascontiguousarray(full.transpose(1, 0, 2)), res


def kernel(**inputs) -> np.ndarray:
    out, _ = _run(T, False, **inputs)
    return out


def kernel_traced(t_steps=T, **inputs):
    out, res = _run(t_steps, True, **inputs)
    return out, res



# revision 12
# speedup vs baseline: 1.2591x; 1.2591x over previous
"""SC-LSTM decoder (2-layer, teacher-forced) Trainium2 Bass kernel — v2.

Strategy (8 NeuronCores, tensor-parallel over H):
  - Core j owns gate columns [128j, 128j+128) of each layer's hidden state
    (GS=512 packed gate cols) and V-cols [256j, 256j+256) of the output
    projection.  Full batch B=128 on every core fills the PE stationary dim.
  - All GEMM inputs are bf16 (PSUM accumulates fp32).  Per gate block the
    x-contribution, h-contributions and (for r) the shared semantic-gate
    terms accumulate into a single PSUM group — the PE does the adds, so
    there is no separate x-precompute phase, no DRAM scratch, and no
    vector-engine gsum/rpre additions.
  - Two bf16 AllGathers per step rebuild the full transposed hidden state
    (32KB/rank each).  AG windows are filled with the output projection of
    the previous step, the next step's x-GEMMs, and the next step's
    precomputable layer-0 gate work.
"""

import sys

sys.path.insert(0, "/opt/trn_rl_repo")

import numpy as np

import concourse.bass as bass
import concourse.mybir as mybir
import concourse.tile as tile
from concourse import bacc
from concourse.bass_utils import run_bass_kernel_spmd
from concourse.masks import make_identity

B, T, E, H, D, V, L = 128, 100, 2048, 1024, 256, 2048, 2
NC = 8
P = 128
HS = H // NC      # 128 h-rows per core per layer
GS = 4 * HS       # 512 packed gate cols per core
VS = V // NC      # 256 output cols per core
KE = E // P       # 16 k-tiles over E
KH = H // P       # 8 k-tiles over H
DK = D // P       # 2 k-tiles over D
XB = 4            # x-tile DMA batch (steps per load)
F32 = mybir.dt.float32
BF16 = mybir.dt.bfloat16

_cache = {}


def _build(t_steps: int):
    nc = bacc.Bacc("TRN2", target_bir_lowering=False, debug=False, num_devices=NC)

    # ---------------- I/O declarations (per-core values supplied via in_maps)
    xT = nc.dram_tensor("xT", [E, t_steps * B], BF16, kind="ExternalInput")
    h0T_i = nc.dram_tensor("h0T_i", [H, B], BF16, kind="ExternalInput")
    c_i = nc.dram_tensor("c_i", [B, HS], F32, kind="ExternalInput")
    d_i = nc.dram_tensor("d_i", [B, D], F32, kind="ExternalInput")
    Wx0 = nc.dram_tensor("Wx0", [E, GS], BF16, kind="ExternalInput")
    Wx1x = nc.dram_tensor("Wx1x", [E, GS], BF16, kind="ExternalInput")
    Wh0 = nc.dram_tensor("Wh0", [H, GS], BF16, kind="ExternalInput")
    Wh1 = nc.dram_tensor("Wh1", [H, GS], BF16, kind="ExternalInput")
    Wx1h = nc.dram_tensor("Wx1h", [H, GS], BF16, kind="ExternalInput")
    Wrx0 = nc.dram_tensor("Wrx0", [E, D], BF16, kind="ExternalInput")
    Wrx1 = nc.dram_tensor("Wrx1", [E, D], BF16, kind="ExternalInput")
    Wrc0 = nc.dram_tensor("Wrc0", [H, D], BF16, kind="ExternalInput")
    Wrc1 = nc.dram_tensor("Wrc1", [H, D], BF16, kind="ExternalInput")
    Wr1h = nc.dram_tensor("Wr1h", [H, D], BF16, kind="ExternalInput")
    Wdc0 = nc.dram_tensor("Wdc0", [D, HS], BF16, kind="ExternalInput")
    Wdc1 = nc.dram_tensor("Wdc1", [D, HS], BF16, kind="ExternalInput")
    Wout = nc.dram_tensor("Wout", [2 * H, VS], BF16, kind="ExternalInput")

    out_o = nc.dram_tensor("out", [t_steps, B, VS], F32, kind="ExternalOutput")

    rg = [list(range(NC))]
    Sig = mybir.ActivationFunctionType.Sigmoid
    Tanh = mybir.ActivationFunctionType.Tanh
    mul = mybir.AluOpType.mult
    add = mybir.AluOpType.add

    with tile.TileContext(nc) as tc:
        with (
            tc.tile_pool(name="const", bufs=1) as constp,
            tc.tile_pool(name="wr", bufs=1) as wrp,
            tc.tile_pool(name="st", bufs=2) as stp,
            tc.tile_pool(name="xb", bufs=2) as xbp,
            tc.tile_pool(name="wk", bufs=2) as wkp,
            tc.tile_pool(name="psg", bufs=1, space="PSUM") as psg,
            tc.tile_pool(name="psr", bufs=1, space="PSUM") as psr,
            tc.tile_pool(name="pso", bufs=1, space="PSUM") as pso,
            tc.tile_pool(name="pst", bufs=2, space="PSUM") as pst,
            tc.tile_pool(name="dma_b", bufs=4, space="DRAM") as dramp,
        ):
            ident = constp.tile([P, P], F32)
            make_identity(nc, ident[:])

            # ---------------- resident weights
            wx0 = wrp.tile([P, KE, GS], BF16)
            wx1x = wrp.tile([P, KE, GS], BF16)
            wh0 = wrp.tile([P, KH, GS], BF16)
            wh1 = wrp.tile([P, KH, GS], BF16)
            wx1h = wrp.tile([P, KH, GS], BF16)
            wrx0 = wrp.tile([P, KE, D], BF16)
            wrx1 = wrp.tile([P, KE, D], BF16)
            wrc0 = wrp.tile([P, KH, D], BF16)
            wrc1 = wrp.tile([P, KH, D], BF16)
            wr1h = wrp.tile([P, KH, D], BF16)
            wdc0 = wrp.tile([P, DK, HS], BF16)
            wdc1 = wrp.tile([P, DK, HS], BF16)
            wout = wrp.tile([P, 2 * KH, VS], BF16)
            for dst, src in [
                (wx0, Wx0), (wx1x, Wx1x), (wh0, Wh0), (wh1, Wh1), (wx1h, Wx1h),
                (wrx0, Wrx0), (wrx1, Wrx1), (wrc0, Wrc0), (wrc1, Wrc1),
                (wr1h, Wr1h), (wdc0, Wdc0), (wdc1, Wdc1), (wout, Wout),
            ]:
                nc.sync.dma_start(dst[:], src.rearrange("(k p) n -> p k n", p=P))

            # ---------------- initial state
            h0T = stp.tile([P, KH, B], BF16, tag="h0T", name="h0Ti")
            h1T = stp.tile([P, KH, B], BF16, tag="h1T", name="h1Ti")
            nc.sync.dma_start(h0T[:], h0T_i.rearrange("(k p) n -> p k n", p=P))
            nc.sync.dma_start(h1T[:], h0T_i.rearrange("(k p) n -> p k n", p=P))
            c0 = stp.tile([B, HS], F32, tag="c0", name="c0i")
            c1 = stp.tile([B, HS], F32, tag="c1", name="c1i")
            nc.sync.dma_start(c0[:], c_i[:])
            nc.sync.dma_start(c1[:], c_i[:])
            d0 = stp.tile([B, D], F32, tag="d0", name="d0i")
            d1 = stp.tile([B, D], F32, tag="d1", name="d1i")
            nc.sync.dma_start(d0[:], d_i[:])
            nc.sync.dma_start(d1[:], d_i[:])

            # ---------------- x tiles (XB steps per DMA)
            def load_xbatch(u0):
                xtb = xbp.tile([P, KE, XB * B], BF16, tag="xtb", name=f"xtb{u0}")
                nb = min(XB, t_steps - u0) * B
                nc.sync.dma_start(
                    xtb[:, :, :nb],
                    xT[:, u0 * B : u0 * B + nb].rearrange("(k p) n -> p k n", p=P),
                )
                return xtb

            xring = [None, None]
            xring[0] = load_xbatch(0)
            if t_steps > XB:
                xring[1] = load_xbatch(XB)

            def xk(t, k):
                """k-tile AP of x for step t."""
                xtb = xring[(t // XB) % 2]
                s = t % XB
                return xtb[:, k, s * B : (s + 1) * B]

            # ---------------- helpers
            def mm(ps, lhsT, rhs, st_, sp_):
                return nc.tensor.matmul(ps, lhsT, rhs, start=st_, stop=sp_)

            def x_parts(t, which):
                """x-contribution k-tiles (start each PSUM group)."""
                if which == "g0":
                    ps = psg.tile([B, GS], F32, tag="g0p", bufs=1, name=f"g0p{t}")
                    w = wx0
                elif which == "g1":
                    ps = psg.tile([B, GS], F32, tag="g1p", bufs=2, name=f"g1p{t}")
                    w = wx1x
                elif which == "r0":
                    ps = psr.tile([B, D], F32, tag="r0p", bufs=1, name=f"r0p{t}")
                    w = wrx0
                else:
                    ps = psr.tile([B, D], F32, tag="r1p", bufs=1, name=f"r1p{t}")
                    w = wrx1
                for k in range(KE):
                    mm(ps[:], xk(t, k), w[:, k, :], k == 0, False)
                return ps

            def h_parts(ps, hT, w, stop):
                for k in range(KH):
                    mm(ps[:], hT[:, k, :], w[:, k, :], False, stop and k == KH - 1)

            def gate_pre(gp, c_cur, li, t):
                """sigmoid/tanh of gate block + candidate-cell partial."""
                sig = wkp.tile([B, 3 * HS], F32, tag=f"sig{li}", name=f"sig{li}_{t}")
                nc.scalar.activation(sig[:], gp[:, : 3 * HS], Sig)
                tgc = wkp.tile([B, HS], F32, tag=f"tgc{li}", name=f"tgc{li}_{t}")
                nc.scalar.activation(tgc[:], gp[:, 3 * HS :], Tanh)
                t1 = wkp.tile([B, HS], F32, tag=f"t1{li}", name=f"t1{li}_{t}")
                nc.vector.tensor_tensor(t1[:], sig[:, :HS], tgc[:], mul)
                m2 = wkp.tile([B, HS], F32, tag=f"m2{li}", name=f"m2{li}_{t}")
                nc.gpsimd.tensor_tensor(m2[:], sig[:, HS : 2 * HS], c_cur[:], mul)
                cpart = wkp.tile([B, HS], F32, tag=f"cp{li}", name=f"cp{li}_{t}")
                nc.vector.tensor_tensor(cpart[:], t1[:], m2[:], add)
                return sig, cpart

            def r_dc_path(rp, d_cur, wdc, li, t):
                sr = wkp.tile([B, D], F32, tag=f"sr{li}", name=f"sr{li}_{t}")
                nc.scalar.activation(sr[:], rp[:], Sig)
                d_new = stp.tile([B, D], F32, tag=f"d{li}", name=f"d{li}_{t}")
                nc.vector.tensor_tensor(d_new[:], sr[:], d_cur[:], mul)
                # scr bank layout: [0:256) dt-transpose, [256:384) dc matmul,
                # [384:512) nh-transpose
                scr = pst.tile([P, 4 * B], F32, tag="scr", name=f"scr{li}_{t}")
                for k in range(DK):
                    nc.tensor.transpose(scr[:, k * B : (k + 1) * B],
                                        d_new[:, k * P : (k + 1) * P], ident[:])
                dtT = wkp.tile([P, DK * B], BF16, tag=f"dtT{li}", name=f"dtT{li}_{t}")
                nc.vector.tensor_copy(dtT[:], scr[:, : DK * B])
                for k in range(DK):
                    mm(scr[:, 2 * B : 3 * B], dtT[:, k * B : (k + 1) * B],
                       wdc[:, k, :], k == 0, k == DK - 1)
                tdc = wkp.tile([B, HS], F32, tag=f"tdc{li}", name=f"tdc{li}_{t}")
                nc.scalar.activation(tdc[:], scr[:, 2 * B : 3 * B], Tanh)
                return tdc, d_new, scr

            def finish_cell(cpart, tdc, sig, li, t):
                c_new = stp.tile([B, HS], F32, tag=f"c{li}", name=f"c{li}_{t}")
                nc.vector.tensor_tensor(c_new[:], cpart[:], tdc[:], add)
                th = wkp.tile([B, HS], F32, tag=f"th{li}", name=f"th{li}_{t}")
                nc.scalar.activation(th[:], c_new[:], Tanh)
                nh = wkp.tile([B, HS], F32, tag=f"nh{li}", name=f"nh{li}_{t}")
                nc.vector.tensor_tensor(nh[:], sig[:, 2 * HS :], th[:], mul)
                return nh, c_new

            def trigger_gather(nh, scr, li, t):
                nc.tensor.transpose(scr[:, 3 * B :], nh[:], ident[:])
                nhT = wkp.tile([P, B], BF16, tag=f"nhT{li}", name=f"nhT{li}_{t}")
                nc.vector.tensor_copy(nhT[:], scr[:, 3 * B :])
                agi = dramp.tile([P, B], BF16, tag=f"agi{li}", name=f"agi{li}_{t}")
                ago = dramp.tile([H, B], BF16, tag=f"ago{li}", addr_space="Shared",
                                 name=f"ago{li}_{t}")
                nc.sync.dma_start(agi[:], nhT[:])
                cc = nc.gpsimd.collective_compute(
                    "AllGather", mybir.AluOpType.bypass, replica_groups=rg,
                    ins=[agi[:]], outs=[ago[:]],
                )
                return ago, cc

            def load_gathered(ago, li, t):
                hT = stp.tile([P, KH, B], BF16, tag=f"h{li}T", name=f"h{li}T_{t}")
                nc.sync.dma_start(hT[:], ago.rearrange("(k p) n -> p k n", p=P))
                return hT

            def out_proj(h0T_, h1T_, t):
                op = pso.tile([B, VS], F32, tag="outp", name=f"outp{t}")
                for k in range(2 * KH):
                    src = h0T_[:, k, :] if k < KH else h1T_[:, k - KH, :]
                    mm(op[:], src, wout[:, k, :], k == 0, k == 2 * KH - 1)
                osb = wkp.tile([B, VS], F32, tag="osb", name=f"osb{t}")
                nc.vector.tensor_copy(osb[:], op[:])
                nc.sync.dma_start(out_o[t], osb[:])

            # ---------------- prologue: step-0 groups (x + h0 parts) + layer-0
            # gate-side precompute
            g0p = x_parts(0, "g0")
            r0p = x_parts(0, "r0")
            r1p = x_parts(0, "r1")
            g1p = x_parts(0, "g1")
            h_parts(g0p, h0T, wh0, stop=True)
            h_parts(r0p, h0T, wrc0, stop=False)
            h_parts(r1p, h0T, wrc0, stop=False)
            sig0, cpart0 = gate_pre(g0p, c0, 0, -1)

            # ---------------- main loop
            for t in range(t_steps):
                last = t == t_steps - 1
                # -- A: r/g h1-dependent parts for step t (h1T = h1(t-1))
                h_parts(r0p, h1T, wrc1, stop=True)
                h_parts(r1p, h1T, wrc1, stop=False)
                h_parts(g1p, h1T, wh1, stop=False)

                # -- C: layer-0 r-path + cell
                tdc0, d0, scr0 = r_dc_path(r0p, d0, wdc0, 0, t)
                nh0, c0 = finish_cell(cpart0, tdc0, sig0, 0, t)
                ago0, _ = trigger_gather(nh0, scr0, 0, t)

                # -- D: AG1 window fillers (h0T/h1T still hold h(t-1) here)
                if t > 0:
                    out_proj(h0T, h1T, t - 1)
                if t % XB == 0 and t > 0 and t + XB < t_steps:
                    xring[(t // XB + 1) % 2] = load_xbatch(t + XB)
                if not last:
                    g0p_n = x_parts(t + 1, "g0")
                    r0p_n = x_parts(t + 1, "r0")

                # -- E/F: consume AG1 -> finish g1/r1
                h0T_new = load_gathered(ago0, 0, t)
                h_parts(r1p, h0T_new, wr1h, stop=True)
                h_parts(g1p, h0T_new, wx1h, stop=True)

                # -- G: layer-1 gates + cell
                sig1, cpart1 = gate_pre(g1p, c1, 1, t)
                tdc1, d1, scr1 = r_dc_path(r1p, d1, wdc1, 1, t)
                nh1, c1 = finish_cell(cpart1, tdc1, sig1, 1, t)
                ago1, _ = trigger_gather(nh1, scr1, 1, t)

                # -- H: AG2 window fillers (next step's x/g0/r h0-parts +
                #       layer-0 gate-side precompute)
                if not last:
                    r1p_n = x_parts(t + 1, "r1")
                    g1p_n = x_parts(t + 1, "g1")
                    h_parts(g0p_n, h0T_new, wh0, stop=True)
                    h_parts(r0p_n, h0T_new, wrc0, stop=False)
                    h_parts(r1p_n, h0T_new, wrc0, stop=False)
                    sig0, cpart0 = gate_pre(g0p_n, c0, 0, t)

                # -- I: consume AG2
                h1T_new = load_gathered(ago1, 1, t)

                h0T, h1T = h0T_new, h1T_new
                if not last:
                    g0p, g1p, r0p, r1p = g0p_n, g1p_n, r0p_n, r1p_n

            out_proj(h0T, h1T, t_steps - 1)

    nc.compile()
    return nc


def _prep_inputs(input_seq, h0, dt0, w2h_W0, w2h_b0, w2h_W1, w2h_b1,
                 w2hr_W0, w2hr_b0, w2hr_W1, w2hr_b1,
                 h2h_W0, h2h_b0, h2h_W1, h2h_b1,
                 h2hr_W0, h2hr_b0, h2hr_W1, h2hr_b1,
                 dc_W0, dc_W1, out_W, out_b, t_steps):
    f = np.float32
    bf = np.dtype("bfloat16") if hasattr(np, "bfloat16") else None
    import ml_dtypes
    bf = ml_dtypes.bfloat16
    for name, b in [("w2h_b0", w2h_b0), ("w2h_b1", w2h_b1), ("w2hr_b0", w2hr_b0),
                    ("w2hr_b1", w2hr_b1), ("h2h_b0", h2h_b0), ("h2h_b1", h2h_b1),
                    ("h2hr_b0", h2hr_b0), ("h2hr_b1", h2hr_b1), ("out_b", out_b)]:
        assert not np.any(np.asarray(b)), f"nonzero bias {name} unsupported"

    # time-step inputs: SOS one-hot at t=0, then input_seq[:, t-1]
    xs = np.empty((t_steps, B, E), f)
    xs[0] = 0.0
    xs[0, :, 0] = 1.0
    xs[1:] = np.asarray(input_seq, f).transpose(1, 0, 2)[: t_steps - 1]
    xT = np.ascontiguousarray(xs.reshape(t_steps * B, E).T).astype(bf)

    h0 = np.asarray(h0, f)
    h0T = np.ascontiguousarray(h0.T).astype(bf)
    dt0 = np.asarray(dt0, f)

    alpha = 1.0 / L
    cvt = lambda a: np.ascontiguousarray(np.asarray(a, f)).astype(bf)

    in_maps = []
    for j in range(NC):
        gc = np.r_[tuple(np.arange(g * H + j * HS, g * H + (j + 1) * HS) for g in range(4))]
        vs = slice(j * VS, (j + 1) * VS)
        in_maps.append({
            "xT": xT,
            "h0T_i": h0T,
            "c_i": np.ascontiguousarray(h0[:, j * HS : (j + 1) * HS]),
            "d_i": dt0,
            "Wx0": cvt(np.asarray(w2h_W0, f)[:, gc]),
            "Wx1x": cvt(np.asarray(w2h_W1, f)[:E, gc]),
            "Wh0": cvt(np.asarray(h2h_W0, f)[:, gc]),
            "Wh1": cvt(np.asarray(h2h_W1, f)[:, gc]),
            "Wx1h": cvt(np.asarray(w2h_W1, f)[E:, gc]),
            "Wrx0": cvt(w2hr_W0),
            "Wrx1": cvt(np.asarray(w2hr_W1, f)[:E]),
            "Wrc0": cvt(np.asarray(h2hr_W0, f) * alpha),
            "Wrc1": cvt(np.asarray(h2hr_W1, f) * alpha),
            "Wr1h": cvt(np.asarray(w2hr_W1, f)[E:]),
            "Wdc0": cvt(np.asarray(dc_W0, f)[:, j * HS : (j + 1) * HS]),
            "Wdc1": cvt(np.asarray(dc_W1, f)[:, j * HS : (j + 1) * HS]),
            "Wout": cvt(np.asarray(out_W, f)[:, vs]),
        })
    return in_maps


def _run(t_steps, trace, **inputs):
    if trace:
        import prof_shim

        prof_shim.install()
    key = t_steps
    if key not in _cache:
        _cache[key] = _build(t_steps)
    nc = _cache[key]
    in_maps = _prep_inputs(**inputs, t_steps=t_steps)
    res = run_bass_kernel_spmd(nc, in_maps, list(range(NC)), trace=trace)
    parts = [res.results[j]["out"] for j in range(NC)]  # each (T, B, VS)
    full = np.concatenate(parts, axis=2)                # (T, B, V)
    return np.ascontiguousarray(full.transpose(1, 0, 2)), res


def kernel(**inputs) -> np.ndarray:
    out, _ = _run(T, False, **inputs)
    return out


def kernel_traced(t_steps=T, **inputs):
    out, res = _run(t_steps, True, **inputs)
    return out, res


# revision 24
# speedup vs baseline: 1.2920x; 1.0262x over previous
"""SC-LSTM decoder (2-layer, teacher-forced) Trainium2 Bass kernel — v2.

Strategy (8 NeuronCores, tensor-parallel over H):
  - Core j owns gate columns [128j, 128j+128) of each layer's hidden state
    (GS=512 packed gate cols) and V-cols [256j, 256j+256) of the output
    projection.  Full batch B=128 on every core fills the PE stationary dim.
  - All GEMM inputs are bf16 (PSUM accumulates fp32).  Per gate block the
    x-contribution, h-contributions and (for r) the shared semantic-gate
    terms accumulate into a single PSUM group — the PE does the adds, so
    there is no separate x-precompute phase, no DRAM scratch, and no
    vector-engine gsum/rpre additions.
  - Two bf16 AllGathers per step rebuild the full transposed hidden state
    (32KB/rank each).  AG windows are filled with the output projection of
    the previous step, the next step's x-GEMMs, and the next step's
    precomputable layer-0 gate work.
"""

import sys

sys.path.insert(0, "/opt/trn_rl_repo")

import numpy as np

import concourse.bass as bass
import concourse.mybir as mybir
import concourse.tile as tile
from concourse import bacc
from concourse.bass_utils import run_bass_kernel_spmd
from concourse.masks import make_identity

B, T, E, H, D, V, L = 128, 100, 2048, 1024, 256, 2048, 2
NC = 8
P = 128
HS = H // NC      # 128 h-rows per core per layer
GS = 4 * HS       # 512 packed gate cols per core
VS = V // NC      # 256 output cols per core
KE = E // P       # 16 k-tiles over E
KH = H // P       # 8 k-tiles over H
DK = D // P       # 2 k-tiles over D
XB = 4            # x-tile DMA batch (steps per load)
F32 = mybir.dt.float32
BF16 = mybir.dt.bfloat16

_cache = {}


def _build(t_steps: int):
    nc = bacc.Bacc("TRN2", target_bir_lowering=False, debug=False, num_devices=NC)

    # ---------------- I/O declarations (per-core values supplied via in_maps)
    xT = nc.dram_tensor("xT", [E, t_steps * B], BF16, kind="ExternalInput")
    h0T_i = nc.dram_tensor("h0T_i", [H, B], BF16, kind="ExternalInput")
    c_i = nc.dram_tensor("c_i", [B, HS], F32, kind="ExternalInput")
    d_i = nc.dram_tensor("d_i", [B, D], F32, kind="ExternalInput")
    Wx0 = nc.dram_tensor("Wx0", [E, GS], BF16, kind="ExternalInput")
    Wx1x = nc.dram_tensor("Wx1x", [E, GS], BF16, kind="ExternalInput")
    Wh0 = nc.dram_tensor("Wh0", [H, GS], BF16, kind="ExternalInput")
    Wh1 = nc.dram_tensor("Wh1", [H, GS], BF16, kind="ExternalInput")
    Wx1h = nc.dram_tensor("Wx1h", [H, GS], BF16, kind="ExternalInput")
    Wrx0 = nc.dram_tensor("Wrx0", [E, D], BF16, kind="ExternalInput")
    Wrx1 = nc.dram_tensor("Wrx1", [E, D], BF16, kind="ExternalInput")
    Wrc0 = nc.dram_tensor("Wrc0", [H, D], BF16, kind="ExternalInput")
    Wrc1 = nc.dram_tensor("Wrc1", [H, D], BF16, kind="ExternalInput")
    Wr1h = nc.dram_tensor("Wr1h", [H, D], BF16, kind="ExternalInput")
    Wdc0 = nc.dram_tensor("Wdc0", [D, HS], BF16, kind="ExternalInput")
    Wdc1 = nc.dram_tensor("Wdc1", [D, HS], BF16, kind="ExternalInput")
    Wout = nc.dram_tensor("Wout", [2 * H, VS], BF16, kind="ExternalInput")

    out_o = nc.dram_tensor("out", [t_steps, B, VS], F32, kind="ExternalOutput")

    rg = [list(range(NC))]
    Sig = mybir.ActivationFunctionType.Sigmoid
    Tanh = mybir.ActivationFunctionType.Tanh
    mul = mybir.AluOpType.mult
    add = mybir.AluOpType.add

    with tile.TileContext(nc) as tc:
        with (
            tc.tile_pool(name="const", bufs=1) as constp,
            tc.tile_pool(name="wr", bufs=1) as wrp,
            tc.tile_pool(name="st", bufs=2) as stp,
            tc.tile_pool(name="xb", bufs=2) as xbp,
            tc.tile_pool(name="wk", bufs=2) as wkp,
            tc.tile_pool(name="psg", bufs=1, space="PSUM") as psg,
            tc.tile_pool(name="psr", bufs=1, space="PSUM") as psr,
            tc.tile_pool(name="pso", bufs=1, space="PSUM") as pso,
            tc.tile_pool(name="pst", bufs=2, space="PSUM") as pst,
            tc.tile_pool(name="dma_b", bufs=4, space="DRAM") as dramp,
        ):
            ident = constp.tile([P, P], F32)
            make_identity(nc, ident[:])

            # ---------------- resident weights
            wx0 = wrp.tile([P, KE, GS], BF16)
            wx1x = wrp.tile([P, KE, GS], BF16)
            wh0 = wrp.tile([P, KH, GS], BF16)
            wh1 = wrp.tile([P, KH, GS], BF16)
            wx1h = wrp.tile([P, KH, GS], BF16)
            wrx0 = wrp.tile([P, KE, D], BF16)
            wrx1 = wrp.tile([P, KE, D], BF16)
            wrc0 = wrp.tile([P, KH, D], BF16)
            wrc1 = wrp.tile([P, KH, D], BF16)
            wr1h = wrp.tile([P, KH, D], BF16)
            wdc0 = wrp.tile([P, DK, HS], BF16)
            wdc1 = wrp.tile([P, DK, HS], BF16)
            wout = wrp.tile([P, 2 * KH, VS], BF16)
            for dst, src in [
                (wx0, Wx0), (wx1x, Wx1x), (wh0, Wh0), (wh1, Wh1), (wx1h, Wx1h),
                (wrx0, Wrx0), (wrx1, Wrx1), (wrc0, Wrc0), (wrc1, Wrc1),
                (wr1h, Wr1h), (wdc0, Wdc0), (wdc1, Wdc1), (wout, Wout),
            ]:
                nc.sync.dma_start(dst[:], src.rearrange("(k p) n -> p k n", p=P))

            # ---------------- initial state
            h0T = stp.tile([P, KH, B], BF16, tag="h0T", name="h0Ti")
            h1T = stp.tile([P, KH, B], BF16, tag="h1T", name="h1Ti")
            nc.sync.dma_start(h0T[:], h0T_i.rearrange("(k p) n -> p k n", p=P))
            nc.sync.dma_start(h1T[:], h0T_i.rearrange("(k p) n -> p k n", p=P))
            c0 = stp.tile([B, HS], F32, tag="c0", name="c0i")
            c1 = stp.tile([B, HS], F32, tag="c1", name="c1i")
            nc.sync.dma_start(c0[:], c_i[:])
            nc.sync.dma_start(c1[:], c_i[:])
            d0 = stp.tile([B, D], F32, tag="d0", name="d0i")
            d1 = stp.tile([B, D], F32, tag="d1", name="d1i")
            nc.sync.dma_start(d0[:], d_i[:])
            nc.sync.dma_start(d1[:], d_i[:])

            # ---------------- x tiles (XB steps per DMA)
            def load_xbatch(u0):
                xtb = xbp.tile([P, KE, XB * B], BF16, tag="xtb", name=f"xtb{u0}")
                nb = min(XB, t_steps - u0) * B
                nc.scalar.dma_start(
                    xtb[:, :, :nb],
                    xT[:, u0 * B : u0 * B + nb].rearrange("(k p) n -> p k n", p=P),
                )
                return xtb

            xring = [None, None]
            xring[0] = load_xbatch(0)
            if t_steps > XB:
                xring[1] = load_xbatch(XB)

            def xk(t, k):
                """k-tile AP of x for step t."""
                xtb = xring[(t // XB) % 2]
                s = t % XB
                return xtb[:, k, s * B : (s + 1) * B]

            # ---------------- helpers
            def mm(ps, lhsT, rhs, st_, sp_):
                return nc.tensor.matmul(ps, lhsT, rhs, start=st_, stop=sp_)

            def x_parts(t, which, pin_after=None):
                """x-contribution k-tiles (start each PSUM group)."""
                if which == "g0":
                    ps = psg.tile([B, GS], F32, tag="g0p", bufs=1, name=f"g0p{t}")
                    w = wx0
                elif which == "g1":
                    ps = psg.tile([B, GS], F32, tag="g1p", bufs=2, name=f"g1p{t}")
                    w = wx1x
                elif which == "r0":
                    ps = psr.tile([B, D], F32, tag="r0p", bufs=1, name=f"r0p{t}")
                    w = wrx0
                else:
                    ps = psr.tile([B, D], F32, tag="r1p", bufs=1, name=f"r1p{t}")
                    w = wrx1
                for k in range(KE):
                    m = mm(ps[:], xk(t, k), w[:, k, :], k == 0, False)
                    if k == 0 and pin_after is not None:
                        bass._add_dep_helper(m.ins, pin_after.ins, sync=True,
                                             reason="pin filler into AG window")
                return ps

            def h_parts(ps, hT, w, stop):
                for k in range(KH):
                    mm(ps[:], hT[:, k, :], w[:, k, :], False, stop and k == KH - 1)

            def gate_pre(gp, c_cur, li, t):
                """sigmoid/tanh of gate block + candidate-cell partial."""
                sig = wkp.tile([B, 3 * HS], F32, tag=f"sig{li}", name=f"sig{li}_{t}")
                nc.scalar.activation(sig[:], gp[:, : 3 * HS], Sig)
                tgc = wkp.tile([B, HS], F32, tag=f"tgc{li}", name=f"tgc{li}_{t}")
                nc.scalar.activation(tgc[:], gp[:, 3 * HS :], Tanh)
                t1 = wkp.tile([B, HS], F32, tag=f"t1{li}", name=f"t1{li}_{t}")
                nc.vector.tensor_tensor(t1[:], sig[:, :HS], tgc[:], mul)
                m2 = wkp.tile([B, HS], F32, tag=f"m2{li}", name=f"m2{li}_{t}")
                nc.vector.tensor_tensor(m2[:], sig[:, HS : 2 * HS], c_cur[:], mul)
                cpart = wkp.tile([B, HS], F32, tag=f"cp{li}", name=f"cp{li}_{t}")
                nc.vector.tensor_tensor(cpart[:], t1[:], m2[:], add)
                return sig, cpart

            def r_dc_path(rp, d_cur, wdc, li, t):
                sr = wkp.tile([B, D], F32, tag=f"sr{li}", name=f"sr{li}_{t}")
                nc.scalar.activation(sr[:], rp[:], Sig)
                d_new = stp.tile([B, D], F32, tag=f"d{li}", name=f"d{li}_{t}")
                nc.vector.tensor_tensor(d_new[:], sr[:], d_cur[:], mul)
                # scr bank layout: [0:256) dt-transpose, [256:384) dc matmul,
                # [384:512) nh-transpose
                scr = pst.tile([P, 4 * B], F32, tag="scr", name=f"scr{li}_{t}")
                for k in range(DK):
                    nc.tensor.transpose(scr[:, k * B : (k + 1) * B],
                                        d_new[:, k * P : (k + 1) * P], ident[:])
                dtT = wkp.tile([P, DK * B], BF16, tag=f"dtT{li}", name=f"dtT{li}_{t}")
                nc.vector.tensor_copy(dtT[:], scr[:, : DK * B])
                for k in range(DK):
                    mm(scr[:, 2 * B : 3 * B], dtT[:, k * B : (k + 1) * B],
                       wdc[:, k, :], k == 0, k == DK - 1)
                tdc = wkp.tile([B, HS], F32, tag=f"tdc{li}", name=f"tdc{li}_{t}")
                nc.scalar.activation(tdc[:], scr[:, 2 * B : 3 * B], Tanh)
                return tdc, d_new, scr

            def finish_cell(cpart, tdc, sig, li, t):
                c_new = stp.tile([B, HS], F32, tag=f"c{li}", name=f"c{li}_{t}")
                nc.vector.tensor_tensor(c_new[:], cpart[:], tdc[:], add)
                th = wkp.tile([B, HS], F32, tag=f"th{li}", name=f"th{li}_{t}")
                nc.scalar.activation(th[:], c_new[:], Tanh)
                nh = wkp.tile([B, HS], F32, tag=f"nh{li}", name=f"nh{li}_{t}")
                nc.vector.tensor_tensor(nh[:], sig[:, 2 * HS :], th[:], mul)
                return nh, c_new

            def trigger_gather(nh, scr, li, t):
                ntr = nc.tensor.transpose(scr[:, 3 * B :], nh[:], ident[:])
                nhT = wkp.tile([P, B], BF16, tag=f"nhT{li}", name=f"nhT{li}_{t}")
                nc.vector.tensor_copy(nhT[:], scr[:, 3 * B :])
                agi = dramp.tile([P, B], BF16, tag=f"agi{li}", name=f"agi{li}_{t}")
                ago = dramp.tile([H, B], BF16, tag=f"ago{li}", addr_space="Shared",
                                 name=f"ago{li}_{t}")
                nc.sync.dma_start(agi[:], nhT[:])
                cc = nc.gpsimd.collective_compute(
                    "AllGather", mybir.AluOpType.bypass, replica_groups=rg,
                    ins=[agi[:]], outs=[ago[:]],
                )
                return ago, ntr

            def load_gathered(ago, li, t):
                hT = stp.tile([P, KH, B], BF16, tag=f"h{li}T", name=f"h{li}T_{t}")
                nc.sync.dma_start(hT[:], ago.rearrange("(k p) n -> p k n", p=P))
                return hT

            def out_proj(h0T_, h1T_, t):
                op = pso.tile([B, VS], F32, tag="outp", name=f"outp{t}")
                m0 = None
                for k in range(2 * KH):
                    src = h0T_[:, k, :] if k < KH else h1T_[:, k - KH, :]
                    m = mm(op[:], src, wout[:, k, :], k == 0, k == 2 * KH - 1)
                    if m0 is None:
                        m0 = m
                osb = wkp.tile([B, VS], F32, tag="osb", name=f"osb{t}")
                nc.vector.tensor_copy(osb[:], op[:])
                nc.scalar.dma_start(out_o[t], osb[:])
                return m0

            # ---------------- prologue: step-0 groups (x + h0 parts) + layer-0
            # gate-side precompute
            g0p = x_parts(0, "g0")
            r0p = x_parts(0, "r0")
            r1p = x_parts(0, "r1")
            g1p = x_parts(0, "g1")
            h_parts(g0p, h0T, wh0, stop=True)
            h_parts(r0p, h0T, wrc0, stop=False)
            h_parts(r1p, h0T, wrc0, stop=False)
            sig0, cpart0 = gate_pre(g0p, c0, 0, -1)

            def pin(inst, after):
                bass._add_dep_helper(inst.ins, after.ins, sync=True,
                                     reason="pin filler into AG window")

            # ---------------- main loop
            for t in range(t_steps):
                last = t == t_steps - 1
                # -- A: r0 h1-part first (h1T = h1(t-1)), then the full
                #       layer-0 critical chain so its notify lands promptly
                h_parts(r0p, h1T, wrc1, stop=True)
                tdc0, d0, scr0 = r_dc_path(r0p, d0, wdc0, 0, t)
                nh0, c0 = finish_cell(cpart0, tdc0, sig0, 0, t)
                ago0, ntr0 = trigger_gather(nh0, scr0, 0, t)

                # -- B: remaining h1-dependent parts (fill the chain's gaps)
                h_parts(r1p, h1T, wrc1, stop=False)
                h_parts(g1p, h1T, wh1, stop=False)

                # -- D: AG1 window fillers, pinned behind the nh0 transpose so
                #       they cannot precede it in the PE FIFO
                if t > 0:
                    m0 = out_proj(h0T, h1T, t - 1)
                    pin(m0, ntr0)
                if t % XB == 0 and t > 0 and t + XB < t_steps:
                    xring[(t // XB + 1) % 2] = load_xbatch(t + XB)
                if not last:
                    g0p_n = x_parts(t + 1, "g0", pin_after=ntr0)
                    r0p_n = x_parts(t + 1, "r0", pin_after=ntr0)

                # -- E/F: consume AG1 -> r1 nh0-part, then layer-1 r-path,
                #         then g1 nh0-part, then gates/cell
                h0T_new = load_gathered(ago0, 0, t)
                h_parts(r1p, h0T_new, wr1h, stop=True)
                tdc1, d1, scr1 = r_dc_path(r1p, d1, wdc1, 1, t)
                h_parts(g1p, h0T_new, wx1h, stop=True)
                sig1, cpart1 = gate_pre(g1p, c1, 1, t)
                nh1, c1 = finish_cell(cpart1, tdc1, sig1, 1, t)
                ago1, ntr1 = trigger_gather(nh1, scr1, 1, t)

                # -- H: AG2 window fillers (next step's x/g0/r h0-parts +
                #       layer-0 gate-side precompute), pinned after nh1 transpose
                if not last:
                    r1p_n = x_parts(t + 1, "r1", pin_after=ntr1)
                    g1p_n = x_parts(t + 1, "g1", pin_after=ntr1)
                    h_parts(g0p_n, h0T_new, wh0, stop=True)
                    h_parts(r0p_n, h0T_new, wrc0, stop=False)
                    h_parts(r1p_n, h0T_new, wrc0, stop=False)
                    sig0, cpart0 = gate_pre(g0p_n, c0, 0, t)

                # -- I: consume AG2
                h1T_new = load_gathered(ago1, 1, t)

                h0T, h1T = h0T_new, h1T_new
                if not last:
                    g0p, g1p, r0p, r1p = g0p_n, g1p_n, r0p_n, r1p_n

            out_proj(h0T, h1T, t_steps - 1)

    nc.compile()
    return nc


def _prep_inputs(input_seq, h0, dt0, w2h_W0, w2h_b0, w2h_W1, w2h_b1,
                 w2hr_W0, w2hr_b0, w2hr_W1, w2hr_b1,
                 h2h_W0, h2h_b0, h2h_W1, h2h_b1,
                 h2hr_W0, h2hr_b0, h2hr_W1, h2hr_b1,
                 dc_W0, dc_W1, out_W, out_b, t_steps):
    f = np.float32
    bf = np.dtype("bfloat16") if hasattr(np, "bfloat16") else None
    import ml_dtypes
    bf = ml_dtypes.bfloat16
    for name, b in [("w2h_b0", w2h_b0), ("w2h_b1", w2h_b1), ("w2hr_b0", w2hr_b0),
                    ("w2hr_b1", w2hr_b1), ("h2h_b0", h2h_b0), ("h2h_b1", h2h_b1),
                    ("h2hr_b0", h2hr_b0), ("h2hr_b1", h2hr_b1), ("out_b", out_b)]:
        assert not np.any(np.asarray(b)), f"nonzero bias {name} unsupported"

    # time-step inputs: SOS one-hot at t=0, then input_seq[:, t-1]
    xs = np.empty((t_steps, B, E), f)
    xs[0] = 0.0
    xs[0, :, 0] = 1.0
    xs[1:] = np.asarray(input_seq, f).transpose(1, 0, 2)[: t_steps - 1]
    xT = np.ascontiguousarray(xs.reshape(t_steps * B, E).T).astype(bf)

    h0 = np.asarray(h0, f)
    h0T = np.ascontiguousarray(h0.T).astype(bf)
    dt0 = np.asarray(dt0, f)

    alpha = 1.0 / L
    cvt = lambda a: np.ascontiguousarray(np.asarray(a, f)).astype(bf)

    in_maps = []
    for j in range(NC):
        gc = np.r_[tuple(np.arange(g * H + j * HS, g * H + (j + 1) * HS) for g in range(4))]
        vs = slice(j * VS, (j + 1) * VS)
        in_maps.append({
            "xT": xT,
            "h0T_i": h0T,
            "c_i": np.ascontiguousarray(h0[:, j * HS : (j + 1) * HS]),
            "d_i": dt0,
            "Wx0": cvt(np.asarray(w2h_W0, f)[:, gc]),
            "Wx1x": cvt(np.asarray(w2h_W1, f)[:E, gc]),
            "Wh0": cvt(np.asarray(h2h_W0, f)[:, gc]),
            "Wh1": cvt(np.asarray(h2h_W1, f)[:, gc]),
            "Wx1h": cvt(np.asarray(w2h_W1, f)[E:, gc]),
            "Wrx0": cvt(w2hr_W0),
            "Wrx1": cvt(np.asarray(w2hr_W1, f)[:E]),
            "Wrc0": cvt(np.asarray(h2hr_W0, f) * alpha),
            "Wrc1": cvt(np.asarray(h2hr_W1, f) * alpha),
            "Wr1h": cvt(np.asarray(w2hr_W1, f)[E:]),
            "Wdc0": cvt(np.asarray(dc_W0, f)[:, j * HS : (j + 1) * HS]),
            "Wdc1": cvt(np.asarray(dc_W1, f)[:, j * HS : (j + 1) * HS]),
            "Wout": cvt(np.asarray(out_W, f)[:, vs]),
        })
    return in_maps


def _run(t_steps, trace, **inputs):
    if trace:
        import prof_shim

        prof_shim.install()
    key = t_steps
    if key not in _cache:
        _cache[key] = _build(t_steps)
    nc = _cache[key]
    in_maps = _prep_inputs(**inputs, t_steps=t_steps)
    res = run_bass_kernel_spmd(nc, in_maps, list(range(NC)), trace=trace)
    parts = [res.results[j]["out"] for j in range(NC)]  # each (T, B, VS)
    full = np.concatenate(parts, axis=2)                # (T, B, V)
    return np.ascontiguousarray(full.transpose(1, 0, 2)), res


def kernel(**inputs) -> np.ndarray:
    out, _ = _run(T, False, **inputs)
    return out


def kernel_traced(t_steps=T, **inputs):
    out, res = _run(t_steps, True, **inputs)
    return out, res


# revision 26
# speedup vs baseline: 1.3157x; 1.0183x over previous
"""SC-LSTM decoder (2-layer, teacher-forced) Trainium2 Bass kernel — v2.

Strategy (8 NeuronCores, tensor-parallel over H):
  - Core j owns gate columns [128j, 128j+128) of each layer's hidden state
    (GS=512 packed gate cols) and V-cols [256j, 256j+256) of the output
    projection.  Full batch B=128 on every core fills the PE stationary dim.
  - All GEMM inputs are bf16 (PSUM accumulates fp32).  Per gate block the
    x-contribution, h-contributions and (for r) the shared semantic-gate
    terms accumulate into a single PSUM group — the PE does the adds, so
    there is no separate x-precompute phase, no DRAM scratch, and no
    vector-engine gsum/rpre additions.
  - Two bf16 AllGathers per step rebuild the full transposed hidden state
    (32KB/rank each).  AG windows are filled with the output projection of
    the previous step, the next step's x-GEMMs, and the next step's
    precomputable layer-0 gate work.
"""

import sys

sys.path.insert(0, "/opt/trn_rl_repo")

import numpy as np

import concourse.bass as bass
import concourse.mybir as mybir
import concourse.tile as tile
from concourse import bacc
from concourse.bass_utils import run_bass_kernel_spmd
from concourse.masks import make_identity

B, T, E, H, D, V, L = 128, 100, 2048, 1024, 256, 2048, 2
NC = 8
P = 128
HS = H // NC      # 128 h-rows per core per layer
GS = 4 * HS       # 512 packed gate cols per core
VS = V // NC      # 256 output cols per core
KE = E // P       # 16 k-tiles over E
KH = H // P       # 8 k-tiles over H
DK = D // P       # 2 k-tiles over D
XB = 4            # x-tile DMA batch (steps per load)
F32 = mybir.dt.float32
BF16 = mybir.dt.bfloat16

_cache = {}


def _build(t_steps: int):
    nc = bacc.Bacc("TRN2", target_bir_lowering=False, debug=False, num_devices=NC)

    # ---------------- I/O declarations (per-core values supplied via in_maps)
    xT = nc.dram_tensor("xT", [E, t_steps * B], BF16, kind="ExternalInput")
    h0T_i = nc.dram_tensor("h0T_i", [H, B], BF16, kind="ExternalInput")
    c_i = nc.dram_tensor("c_i", [B, HS], F32, kind="ExternalInput")
    d_i = nc.dram_tensor("d_i", [B, D], F32, kind="ExternalInput")
    Wx0 = nc.dram_tensor("Wx0", [E, GS], BF16, kind="ExternalInput")
    Wx1x = nc.dram_tensor("Wx1x", [E, GS], BF16, kind="ExternalInput")
    Wh0 = nc.dram_tensor("Wh0", [H, GS], BF16, kind="ExternalInput")
    Wh1 = nc.dram_tensor("Wh1", [H, GS], BF16, kind="ExternalInput")
    Wx1h = nc.dram_tensor("Wx1h", [H, GS], BF16, kind="ExternalInput")
    Wrx0 = nc.dram_tensor("Wrx0", [E, D], BF16, kind="ExternalInput")
    Wrx1 = nc.dram_tensor("Wrx1", [E, D], BF16, kind="ExternalInput")
    Wrc0 = nc.dram_tensor("Wrc0", [H, D], BF16, kind="ExternalInput")
    Wrc1 = nc.dram_tensor("Wrc1", [H, D], BF16, kind="ExternalInput")
    Wr1h = nc.dram_tensor("Wr1h", [H, D], BF16, kind="ExternalInput")
    Wdc0 = nc.dram_tensor("Wdc0", [D, HS], BF16, kind="ExternalInput")
    Wdc1 = nc.dram_tensor("Wdc1", [D, HS], BF16, kind="ExternalInput")
    Wout = nc.dram_tensor("Wout", [2 * H, VS], BF16, kind="ExternalInput")

    out_o = nc.dram_tensor("out", [t_steps, B, VS], F32, kind="ExternalOutput")

    rg = [list(range(NC))]
    Sig = mybir.ActivationFunctionType.Sigmoid
    Tanh = mybir.ActivationFunctionType.Tanh
    mul = mybir.AluOpType.mult
    add = mybir.AluOpType.add

    with tile.TileContext(nc) as tc:
        with (
            tc.tile_pool(name="const", bufs=1) as constp,
            tc.tile_pool(name="wr", bufs=1) as wrp,
            tc.tile_pool(name="st", bufs=2) as stp,
            tc.tile_pool(name="xb", bufs=2) as xbp,
            tc.tile_pool(name="wk", bufs=2) as wkp,
            tc.tile_pool(name="psg", bufs=1, space="PSUM") as psg,
            tc.tile_pool(name="psr", bufs=1, space="PSUM") as psr,
            tc.tile_pool(name="pso", bufs=1, space="PSUM") as pso,
            tc.tile_pool(name="pst", bufs=2, space="PSUM") as pst,
            tc.tile_pool(name="dma_b", bufs=4, space="DRAM") as dramp,
        ):
            ident = constp.tile([P, P], F32)
            make_identity(nc, ident[:])

            # ---------------- resident weights
            wx0 = wrp.tile([P, KE, GS], BF16)
            wx1x = wrp.tile([P, KE, GS], BF16)
            wh0 = wrp.tile([P, KH, GS], BF16)
            wh1 = wrp.tile([P, KH, GS], BF16)
            wx1h = wrp.tile([P, KH, GS], BF16)
            wrx0 = wrp.tile([P, KE, D], BF16)
            wrx1 = wrp.tile([P, KE, D], BF16)
            wrc0 = wrp.tile([P, KH, D], BF16)
            wrc1 = wrp.tile([P, KH, D], BF16)
            wr1h = wrp.tile([P, KH, D], BF16)
            wdc0 = wrp.tile([P, DK, HS], BF16)
            wdc1 = wrp.tile([P, DK, HS], BF16)
            wout = wrp.tile([P, 2 * KH, VS], BF16)
            for dst, src in [
                (wx0, Wx0), (wx1x, Wx1x), (wh0, Wh0), (wh1, Wh1), (wx1h, Wx1h),
                (wrx0, Wrx0), (wrx1, Wrx1), (wrc0, Wrc0), (wrc1, Wrc1),
                (wr1h, Wr1h), (wdc0, Wdc0), (wdc1, Wdc1), (wout, Wout),
            ]:
                nc.sync.dma_start(dst[:], src.rearrange("(k p) n -> p k n", p=P))

            # ---------------- initial state
            h0T = stp.tile([P, KH, B], BF16, tag="h0T", name="h0Ti")
            h1T = stp.tile([P, KH, B], BF16, tag="h1T", name="h1Ti")
            nc.sync.dma_start(h0T[:], h0T_i.rearrange("(k p) n -> p k n", p=P))
            nc.sync.dma_start(h1T[:], h0T_i.rearrange("(k p) n -> p k n", p=P))
            c0 = stp.tile([B, HS], F32, tag="c0", name="c0i")
            c1 = stp.tile([B, HS], F32, tag="c1", name="c1i")
            nc.sync.dma_start(c0[:], c_i[:])
            nc.sync.dma_start(c1[:], c_i[:])
            d0 = stp.tile([B, D], F32, tag="d0", name="d0i")
            d1 = stp.tile([B, D], F32, tag="d1", name="d1i")
            nc.sync.dma_start(d0[:], d_i[:])
            nc.sync.dma_start(d1[:], d_i[:])

            # ---------------- x tiles (XB steps per DMA)
            def load_xbatch(u0):
                xtb = xbp.tile([P, KE, XB * B], BF16, tag="xtb", name=f"xtb{u0}")
                nb = min(XB, t_steps - u0) * B
                nc.scalar.dma_start(
                    xtb[:, :, :nb],
                    xT[:, u0 * B : u0 * B + nb].rearrange("(k p) n -> p k n", p=P),
                )
                return xtb

            xring = [None, None]
            xring[0] = load_xbatch(0)
            if t_steps > XB:
                xring[1] = load_xbatch(XB)

            def xk(t, k):
                """k-tile AP of x for step t."""
                xtb = xring[(t // XB) % 2]
                s = t % XB
                return xtb[:, k, s * B : (s + 1) * B]

            # ---------------- helpers
            def mm(ps, lhsT, rhs, st_, sp_):
                return nc.tensor.matmul(ps, lhsT, rhs, start=st_, stop=sp_)

            def x_parts(t, which, pin_after=None):
                """x-contribution k-tiles (start each PSUM group)."""
                if which == "g0":
                    ps = psg.tile([B, GS], F32, tag="g0p", bufs=1, name=f"g0p{t}")
                    w = wx0
                elif which == "g1":
                    ps = psg.tile([B, GS], F32, tag="g1p", bufs=2, name=f"g1p{t}")
                    w = wx1x
                elif which == "r0":
                    ps = psr.tile([B, D], F32, tag="r0p", bufs=1, name=f"r0p{t}")
                    w = wrx0
                else:
                    ps = psr.tile([B, D], F32, tag="r1p", bufs=1, name=f"r1p{t}")
                    w = wrx1
                for k in range(KE):
                    m = mm(ps[:], xk(t, k), w[:, k, :], k == 0, False)
                    if k == 0 and pin_after is not None:
                        bass._add_dep_helper(m.ins, pin_after.ins, sync=True,
                                             reason="pin filler into AG window")
                return ps

            def h_parts(ps, hT, w, stop, pin_after=None):
                for k in range(KH):
                    m = mm(ps[:], hT[:, k, :], w[:, k, :], False, stop and k == KH - 1)
                    if k == 0 and pin_after is not None:
                        bass._add_dep_helper(m.ins, pin_after.ins, sync=True,
                                             reason="pin filler into AG window")

            def gate_pre(gp, c_cur, li, t):
                """sigmoid/tanh of gate block + candidate-cell partial."""
                sig = wkp.tile([B, 3 * HS], F32, tag=f"sig{li}", name=f"sig{li}_{t}")
                nc.scalar.activation(sig[:], gp[:, : 3 * HS], Sig)
                tgc = wkp.tile([B, HS], F32, tag=f"tgc{li}", name=f"tgc{li}_{t}")
                nc.scalar.activation(tgc[:], gp[:, 3 * HS :], Tanh)
                t1 = wkp.tile([B, HS], F32, tag=f"t1{li}", name=f"t1{li}_{t}")
                nc.vector.tensor_tensor(t1[:], sig[:, :HS], tgc[:], mul)
                m2 = wkp.tile([B, HS], F32, tag=f"m2{li}", name=f"m2{li}_{t}")
                nc.vector.tensor_tensor(m2[:], sig[:, HS : 2 * HS], c_cur[:], mul)
                cpart = wkp.tile([B, HS], F32, tag=f"cp{li}", name=f"cp{li}_{t}")
                nc.vector.tensor_tensor(cpart[:], t1[:], m2[:], add)
                return sig, cpart

            def r_dc_path(rp, d_cur, wdc, li, t):
                sr = wkp.tile([B, D], F32, tag=f"sr{li}", name=f"sr{li}_{t}")
                nc.scalar.activation(sr[:], rp[:], Sig)
                d_new = stp.tile([B, D], F32, tag=f"d{li}", name=f"d{li}_{t}")
                nc.vector.tensor_tensor(d_new[:], sr[:], d_cur[:], mul)
                # scr bank layout: [0:256) dt-transpose, [256:384) dc matmul,
                # [384:512) nh-transpose
                scr = pst.tile([P, 4 * B], F32, tag="scr", name=f"scr{li}_{t}")
                for k in range(DK):
                    nc.tensor.transpose(scr[:, k * B : (k + 1) * B],
                                        d_new[:, k * P : (k + 1) * P], ident[:])
                dtT = wkp.tile([P, DK * B], BF16, tag=f"dtT{li}", name=f"dtT{li}_{t}")
                nc.vector.tensor_copy(dtT[:], scr[:, : DK * B])
                for k in range(DK):
                    mm(scr[:, 2 * B : 3 * B], dtT[:, k * B : (k + 1) * B],
                       wdc[:, k, :], k == 0, k == DK - 1)
                tdc = wkp.tile([B, HS], F32, tag=f"tdc{li}", name=f"tdc{li}_{t}")
                nc.scalar.activation(tdc[:], scr[:, 2 * B : 3 * B], Tanh)
                return tdc, d_new, scr

            def finish_cell(cpart, tdc, sig, li, t):
                c_new = stp.tile([B, HS], F32, tag=f"c{li}", name=f"c{li}_{t}")
                nc.vector.tensor_tensor(c_new[:], cpart[:], tdc[:], add)
                th = wkp.tile([B, HS], F32, tag=f"th{li}", name=f"th{li}_{t}")
                nc.scalar.activation(th[:], c_new[:], Tanh)
                nh = wkp.tile([B, HS], F32, tag=f"nh{li}", name=f"nh{li}_{t}")
                nc.vector.tensor_tensor(nh[:], sig[:, 2 * HS :], th[:], mul)
                return nh, c_new

            def trigger_gather(nh, scr, li, t):
                ntr = nc.tensor.transpose(scr[:, 3 * B :], nh[:], ident[:])
                nhT = wkp.tile([P, B], BF16, tag=f"nhT{li}", name=f"nhT{li}_{t}")
                nc.vector.tensor_copy(nhT[:], scr[:, 3 * B :])
                agi = dramp.tile([P, B], BF16, tag=f"agi{li}", name=f"agi{li}_{t}")
                ago = dramp.tile([H, B], BF16, tag=f"ago{li}", addr_space="Shared",
                                 name=f"ago{li}_{t}")
                nc.sync.dma_start(agi[:], nhT[:])
                cc = nc.gpsimd.collective_compute(
                    "AllGather", mybir.AluOpType.bypass, replica_groups=rg,
                    ins=[agi[:]], outs=[ago[:]],
                )
                return ago, ntr

            def load_gathered(ago, li, t):
                hT = stp.tile([P, KH, B], BF16, tag=f"h{li}T", name=f"h{li}T_{t}")
                nc.sync.dma_start(hT[:], ago.rearrange("(k p) n -> p k n", p=P))
                return hT

            def out_proj(h0T_, h1T_, t):
                op = pso.tile([B, VS], F32, tag="outp", name=f"outp{t}")
                m0 = None
                for k in range(2 * KH):
                    src = h0T_[:, k, :] if k < KH else h1T_[:, k - KH, :]
                    m = mm(op[:], src, wout[:, k, :], k == 0, k == 2 * KH - 1)
                    if m0 is None:
                        m0 = m
                osb = wkp.tile([B, VS], F32, tag="osb", name=f"osb{t}")
                nc.vector.tensor_copy(osb[:], op[:])
                nc.scalar.dma_start(out_o[t], osb[:])
                return m0

            # ---------------- prologue: step-0 groups (x + h0 parts) + layer-0
            # gate-side precompute
            g0p = x_parts(0, "g0")
            r0p = x_parts(0, "r0")
            r1p = x_parts(0, "r1")
            g1p = x_parts(0, "g1")
            h_parts(g0p, h0T, wh0, stop=True)
            h_parts(r0p, h0T, wrc0, stop=False)
            h_parts(r1p, h0T, wrc0, stop=False)
            sig0, cpart0 = gate_pre(g0p, c0, 0, -1)

            def pin(inst, after):
                bass._add_dep_helper(inst.ins, after.ins, sync=True,
                                     reason="pin filler into AG window")

            # ---------------- main loop
            for t in range(t_steps):
                last = t == t_steps - 1
                # -- A: r0 h1-part first (h1T = h1(t-1)), then the full
                #       layer-0 critical chain so its notify lands promptly
                h_parts(r0p, h1T, wrc1, stop=True)
                tdc0, d0, scr0 = r_dc_path(r0p, d0, wdc0, 0, t)
                nh0, c0 = finish_cell(cpart0, tdc0, sig0, 0, t)
                ago0, ntr0 = trigger_gather(nh0, scr0, 0, t)

                # -- B: remaining h1-dependent parts, pushed into the AG1
                #       window so they don't contend with the layer-0 chain
                h_parts(r1p, h1T, wrc1, stop=False, pin_after=ntr0)
                h_parts(g1p, h1T, wh1, stop=False, pin_after=ntr0)

                # -- D: AG1 window fillers, pinned behind the nh0 transpose so
                #       they cannot precede it in the PE FIFO
                if t > 0:
                    m0 = out_proj(h0T, h1T, t - 1)
                    pin(m0, ntr0)
                if t % XB == 0 and t > 0 and t + XB < t_steps:
                    xring[(t // XB + 1) % 2] = load_xbatch(t + XB)
                if not last:
                    g0p_n = x_parts(t + 1, "g0", pin_after=ntr0)
                    r0p_n = x_parts(t + 1, "r0", pin_after=ntr0)

                # -- E/F: consume AG1 -> r1 nh0-part, then layer-1 r-path,
                #         then g1 nh0-part, then gates/cell
                h0T_new = load_gathered(ago0, 0, t)
                h_parts(r1p, h0T_new, wr1h, stop=True)
                tdc1, d1, scr1 = r_dc_path(r1p, d1, wdc1, 1, t)
                h_parts(g1p, h0T_new, wx1h, stop=True)
                sig1, cpart1 = gate_pre(g1p, c1, 1, t)
                nh1, c1 = finish_cell(cpart1, tdc1, sig1, 1, t)
                ago1, ntr1 = trigger_gather(nh1, scr1, 1, t)

                # -- H: AG2 window fillers (next step's x/g0/r h0-parts +
                #       layer-0 gate-side precompute), pinned after nh1 transpose
                if not last:
                    r1p_n = x_parts(t + 1, "r1", pin_after=ntr1)
                    g1p_n = x_parts(t + 1, "g1", pin_after=ntr1)
                    h_parts(g0p_n, h0T_new, wh0, stop=True)
                    h_parts(r0p_n, h0T_new, wrc0, stop=False)
                    h_parts(r1p_n, h0T_new, wrc0, stop=False)
                    sig0, cpart0 = gate_pre(g0p_n, c0, 0, t)

                # -- I: consume AG2
                h1T_new = load_gathered(ago1, 1, t)

                h0T, h1T = h0T_new, h1T_new
                if not last:
                    g0p, g1p, r0p, r1p = g0p_n, g1p_n, r0p_n, r1p_n

            out_proj(h0T, h1T, t_steps - 1)

    nc.compile()
    return nc


def _prep_inputs(input_seq, h0, dt0, w2h_W0, w2h_b0, w2h_W1, w2h_b1,
                 w2hr_W0, w2hr_b0, w2hr_W1, w2hr_b1,
                 h2h_W0, h2h_b0, h2h_W1, h2h_b1,
                 h2hr_W0, h2hr_b0, h2hr_W1, h2hr_b1,
                 dc_W0, dc_W1, out_W, out_b, t_steps):
    f = np.float32
    bf = np.dtype("bfloat16") if hasattr(np, "bfloat16") else None
    import ml_dtypes
    bf = ml_dtypes.bfloat16
    for name, b in [("w2h_b0", w2h_b0), ("w2h_b1", w2h_b1), ("w2hr_b0", w2hr_b0),
                    ("w2hr_b1", w2hr_b1), ("h2h_b0", h2h_b0), ("h2h_b1", h2h_b1),
                    ("h2hr_b0", h2hr_b0), ("h2hr_b1", h2hr_b1), ("out_b", out_b)]:
        assert not np.any(np.asarray(b)), f"nonzero bias {name} unsupported"

    # time-step inputs: SOS one-hot at t=0, then input_seq[:, t-1]
    xs = np.empty((t_steps, B, E), f)
    xs[0] = 0.0
    xs[0, :, 0] = 1.0
    xs[1:] = np.asarray(input_seq, f).transpose(1, 0, 2)[: t_steps - 1]
    xT = np.ascontiguousarray(xs.reshape(t_steps * B, E).T).astype(bf)

    h0 = np.asarray(h0, f)
    h0T = np.ascontiguousarray(h0.T).astype(bf)
    dt0 = np.asarray(dt0, f)

    alpha = 1.0 / L
    cvt = lambda a: np.ascontiguousarray(np.asarray(a, f)).astype(bf)

    in_maps = []
    for j in range(NC):
        gc = np.r_[tuple(np.arange(g * H + j * HS, g * H + (j + 1) * HS) for g in range(4))]
        vs = slice(j * VS, (j + 1) * VS)
        in_maps.append({
            "xT": xT,
            "h0T_i": h0T,
            "c_i": np.ascontiguousarray(h0[:, j * HS : (j + 1) * HS]),
            "d_i": dt0,
            "Wx0": cvt(np.asarray(w2h_W0, f)[:, gc]),
            "Wx1x": cvt(np.asarray(w2h_W1, f)[:E, gc]),
            "Wh0": cvt(np.asarray(h2h_W0, f)[:, gc]),
            "Wh1": cvt(np.asarray(h2h_W1, f)[:, gc]),
            "Wx1h": cvt(np.asarray(w2h_W1, f)[E:, gc]),
            "Wrx0": cvt(w2hr_W0),
            "Wrx1": cvt(np.asarray(w2hr_W1, f)[:E]),
            "Wrc0": cvt(np.asarray(h2hr_W0, f) * alpha),
            "Wrc1": cvt(np.asarray(h2hr_W1, f) * alpha),
            "Wr1h": cvt(np.asarray(w2hr_W1, f)[E:]),
            "Wdc0": cvt(np.asarray(dc_W0, f)[:, j * HS : (j + 1) * HS]),
            "Wdc1": cvt(np.asarray(dc_W1, f)[:, j * HS : (j + 1) * HS]),
            "Wout": cvt(np.asarray(out_W, f)[:, vs]),
        })
    return in_maps


def _run(t_steps, trace, **inputs):
    if trace:
        import prof_shim

        prof_shim.install()
    key = t_steps
    if key not in _cache:
        _cache[key] = _build(t_steps)
    nc = _cache[key]
    in_maps = _prep_inputs(**inputs, t_steps=t_steps)
    res = run_bass_kernel_spmd(nc, in_maps, list(range(NC)), trace=trace)
    parts = [res.results[j]["out"] for j in range(NC)]  # each (T, B, VS)
    full = np.concatenate(parts, axis=2)                # (T, B, V)
    return np.ascontiguousarray(full.transpose(1, 0, 2)), res


def kernel(**inputs) -> np.ndarray:
    out, _ = _run(T, False, **inputs)
    return out


def kernel_traced(t_steps=T, **inputs):
    out, res = _run(t_steps, True, **inputs)
    return out, res
